# revision 28
# baseline (speedup 1.0000x reference)
"""Trainium2 Bass kernel for the nn_Decoder LSTM-decoder problem.

Reference computation (per agent, 12 steps):
    gates = dec_in @ w_ih.T + h @ w_hh.T + (b_ih + b_hh)
    i, f, g, o = split(gates); c = sig(f)*c + sig(i)*tanh(g); h = sig(o)*tanh(c)
    rel = h @ w_hp.T + b_hp; dec_in = rel @ w_se.T + b_se
Output: rel per step, [12, N, 2].

Algebraic fusion: dec_in_t is linear in h_t, so for steps >= 2
    gates_t = h_{t-1} @ W_eff.T + b_eff,  W_eff = w_hh + w_ih @ w_se @ w_hp
and step 1 uses w_hh plus U = (w_ih @ w_se) applied to last_pos_rel.
last_pos is dead (never affects the output).

Distribution: pure data parallel, 8192 agents per core on 8 NeuronCores.

The Scalar engine (ACT) is the roofline: 5 LUT passes per (agent, hidden,
step) = 491520 FD-columns per core at 1 col/cycle @ 1.2 GHz (~410us) plus
~352 cycles fixed overhead per ACTIVATE. Design choices:
  - Units of 2048 agents; PSUM = 2 rotating slots of [128, 2048] fp32
    (4 banks each); gate ACT ops are FD=2048 (matmuls FD=512, the ISA cap).
  - Gates are processed [g, f, i, o] and the ACT stream per unit is
    [tanh_g, sig_f, sig_i, sig_o, tanh_c-half0, tanh_c-half1]: tanh_g
    first makes the m1/m2/c-add DVE chain finish early, and the cell
    update + tanh(c) run in FD-1024 halves so tanh_c-half0 is ready
    before sig_o retires -> the ACT engine streams with ~zero bubbles
    while the next unit's matmuls recycle the PSUM slots (important
    because the PE HAM clock-gate runs the PE at 1.2 GHz half the time).
  - rel = w_hp.T @ h is deferred two units and runs at the unit tail into
    the 5th PSUM alloc of the pool rotation; raw rel is copied to SBUF
    once and DMA'd to separate x/y DRAM planes; b_hp is added on the
    host, which also interleaves x/y into the [T, N, 2] output.
  - h0/c0/last_pos_rel are pre-transposed and bf16-cast on the HOST, so
    the prologue is 3 plain DMAs per unit (no on-device transposes).
"""

import sys

if "/opt/trn_rl_repo" not in sys.path:
    sys.path.insert(0, "/opt/trn_rl_repo")

import numpy as np

T = 12          # steps
H = 128         # hidden dim
NCORES = 8
NPC = 8192      # agents per core
CH = 2048       # agents per unit (one gate tile = 4 PSUM banks at fp32)

REL_COLTILE = False

_CACHE = {}


def _build_program(npc):
    import concourse.bass as bass
    import concourse.tile as tile
    from concourse import bacc, mybir

    dt = mybir.dt
    f32 = dt.float32
    bf16 = dt.bfloat16
    Act = mybir.ActivationFunctionType

    nsc = npc // CH
    assert npc % CH == 0

    nc = bacc.Bacc(
        "TRN2",
        target_bir_lowering=False,
        debug=False,
        num_devices=NCORES,
    )

    def din(name, shape, dt_=None):
        return nc.dram_tensor(
            name, list(shape), dt_ or f32, kind="ExternalInput"
        ).ap()

    # host-pretransposed bf16 states
    h0T_d = din("h0T", [H, npc], bf16)
    c0T_d = din("c0T", [H, npc], bf16)
    lprT_d = din("lprT", [2, npc], bf16)
    # lhsT layouts, K on partitions. Gate order [i, f, g, o] (torch order).
    wg_d = din("wg", [H, 4 * H], bf16)   # W_eff.T column blocks per gate
    whh_d = din("whh", [H, 4 * H], bf16)  # w_hh.T (step 1)
    u_d = din("u", [2, 4 * H], bf16)      # (w_ih @ w_se).T (step 1)
    bias_d = din("bias", [H, 8])          # ACT bias: [b_eff | b1] x [i,f,g,o]
    whp_d = din("whp", [H, 2], bf16)      # w_hp.T
    outx_d = nc.dram_tensor("outx", [T, npc], f32, kind="ExternalOutput").ap()
    outy_d = nc.dram_tensor("outy", [T, npc], f32, kind="ExternalOutput").ap()

    with tile.TileContext(nc) as tc:
        with (
            tc.tile_pool(name="wpool", bufs=1) as wp,
            tc.tile_pool(name="state", bufs=1) as state,
            tc.tile_pool(name="sig", bufs=2) as sigp,
            tc.tile_pool(name="tmp", bufs=2) as tmpp,
            tc.tile_pool(name="rels", bufs=2) as relp,
            tc.tile_pool(name="ps", bufs=2, space="PSUM") as psp,
        ):
            def wtile(ap, shape, tag, dt_=None):
                t_ = wp.tile(list(shape), dt_ or f32, tag=tag)
                nc.sync.dma_start(t_[:], ap)
                return t_

            wg = wtile(wg_d, [H, 4 * H], "wg", bf16)
            whh = wtile(whh_d, [H, 4 * H], "whh", bf16)
            u = wtile(u_d, [2, 4 * H], "u", bf16)
            bias = wtile(bias_d, [H, 8], "bias")
            whp = wtile(whp_d, [H, 2], "whp", bf16)

            h_sb = state.tile([H, npc], bf16, tag="h")
            c_sb = state.tile([H, npc], bf16, tag="c")
            lpr_sb = state.tile([2, npc], bf16, tag="lpr")

            units = [(t, sc) for t in range(T) for sc in range(nsc)]
            pend_back = []  # [(t, sc, so)] awaiting tanh_c + h update
            pend_rel = []   # [(t, sc)] awaiting rel matmul + writeback

            def emit_rel(t_, sc_):
                """rel = w_hp.T @ h into the 5th PSUM alloc of this unit;
                col-tiled so one unit's rel occupies a single PSUM bank."""
                if REL_COLTILE:
                    rp = psp.tile([128, CH], f32, tag="ps",
                                  name=f"rel{t_}_{sc_}")
                    for q in range(4):
                        hs = slice(sc_ * CH + q * 512,
                                   sc_ * CH + (q + 1) * 512)
                        nc.tensor.matmul(
                            rp[32 * q:32 * q + 2, 0:512], whp[:],
                            h_sb[:, hs], start=True, stop=True,
                            tile_position=(0, 32 * q))
                    ex_x = relp.tile([4, 512], f32, tag="exx")
                    ex_y = relp.tile([4, 512], f32, tag="exy")
                    nc.vector.tensor_copy(ex_x[:], rp[0:97:32, 0:512])
                    nc.vector.tensor_copy(ex_y[:], rp[1:98:32, 0:512])
                    cols = slice(sc_ * CH, (sc_ + 1) * CH)
                    nc.sync.dma_start(outx_d[t_, cols], ex_x[:])
                    nc.sync.dma_start(outy_d[t_, cols], ex_y[:])
                else:
                    rp = psp.tile([128, CH], f32, tag="ps",
                                  name=f"rel{t_}_{sc_}")
                    for q in range(4):
                        osl = slice(q * 512, (q + 1) * 512)
                        hs = slice(sc_ * CH + q * 512,
                                   sc_ * CH + (q + 1) * 512)
                        nc.tensor.matmul(
                            rp[0:2, osl], whp[:], h_sb[:, hs],
                            start=True, stop=True)
                    ex = relp.tile([2, CH], f32, tag="ex")
                    nc.vector.tensor_copy(ex[:], rp[0:2, :])
                    cols = slice(sc_ * CH, (sc_ + 1) * CH)
                    nc.sync.dma_start(outx_d[t_, cols], ex[0:1, :])
                    nc.sync.dma_start(outy_d[t_, cols], ex[1:2, :])

            for u_idx, (t, sc) in enumerate(units):
                cols = slice(sc * CH, (sc + 1) * CH)
                first = t == 0

                if first:
                    nc.sync.dma_start(h_sb[:, cols], h0T_d[:, cols])
                    nc.sync.dma_start(c_sb[:, cols], c0T_d[:, cols])
                    nc.sync.dma_start(lpr_sb[:, cols], lprT_d[:, cols])

                # gate matmuls in processing order [g, f, i, o] (tanh gate
                # FIRST so the m2->cadd->tanh_c chain completes by unit end)
                gt = {}
                for g in (2, 1, 0, 3):
                    pt = psp.tile([128, CH], f32, tag="ps", name=f"g{g}")
                    wsl = slice(g * H, (g + 1) * H)
                    for q in range(4):
                        osl = slice(q * 512, (q + 1) * 512)
                        hs = slice(sc * CH + q * 512,
                                   sc * CH + (q + 1) * 512)
                        if first:
                            nc.tensor.matmul(
                                pt[:, osl], whh[:, wsl], h_sb[:, hs],
                                start=True, stop=False)
                            nc.tensor.matmul(
                                pt[:, osl], u[:, wsl], lpr_sb[:, hs],
                                start=False, stop=True)
                        else:
                            nc.tensor.matmul(
                                pt[:, osl], wg[:, wsl], h_sb[:, hs],
                                start=True, stop=True)
                    gt[g] = pt

                # gate activations (bias fused; cols 4..7 hold step-1 biases)
                bcol = 4 if first else 0
                tg = sigp.tile([128, CH], bf16, tag="tg")
                sf = sigp.tile([128, CH], bf16, tag="sf")
                si = sigp.tile([128, CH], bf16, tag="si")
                so = sigp.tile([128, CH], bf16, tag="so")
                nc.scalar.activation(tg[:], gt[2][:], Act.Tanh,
                                     bias=bias[:, bcol + 2:bcol + 3])
                nc.scalar.activation(sf[:], gt[1][:], Act.Sigmoid,
                                     bias=bias[:, bcol + 1:bcol + 2])
                nc.scalar.activation(si[:], gt[0][:], Act.Sigmoid,
                                     bias=bias[:, bcol:bcol + 1])
                nc.scalar.activation(so[:], gt[3][:], Act.Sigmoid,
                                     bias=bias[:, bcol + 3:bcol + 4])

                # DVE cell update in FD-1024 halves so the first tanh(c)
                # half is ready before sig_o finishes -> gapless ACT stream.
                m1 = tmpp.tile([128, CH], bf16, tag="m1")
                m2 = tmpp.tile([128, CH], bf16, tag="m2")
                tcl = sigp.tile([128, CH], bf16, tag="tc")
                for hf in range(2):
                    hsl = slice(hf * 1024, (hf + 1) * 1024)
                    csl = slice(sc * CH + hf * 1024, sc * CH + (hf + 1) * 1024)
                    nc.vector.tensor_mul(m1[:, hsl], sf[:, hsl],
                                         c_sb[:, csl])
                    nc.vector.tensor_mul(m2[:, hsl], si[:, hsl], tg[:, hsl])
                    nc.vector.tensor_add(c_sb[:, csl], m1[:, hsl],
                                         m2[:, hsl])
                    nc.scalar.activation(tcl[:, hsl], c_sb[:, csl], Act.Tanh)
                pend_back.append((t, sc, so, tcl))

                # rel for the unit two back (h final; slot free after tanh_g)
                pend_rel.append((t, sc))
                if len(pend_rel) > 2:
                    emit_rel(*pend_rel.pop(0))

                # h update (deferred DVE mul; h_prev consumers are >=2
                # units away so this can run late without stalling anyone)
                if len(pend_back) > 1:
                    t_, sc_, so_, tcl_ = pend_back.pop(0)
                    pcols = slice(sc_ * CH, (sc_ + 1) * CH)
                    nc.vector.tensor_mul(h_sb[:, pcols], so_[:], tcl_[:])

            while pend_back:
                t_, sc_, so_, tcl_ = pend_back.pop(0)
                pcols = slice(sc_ * CH, (sc_ + 1) * CH)
                nc.vector.tensor_mul(h_sb[:, pcols], so_[:], tcl_[:])
            while pend_rel:
                emit_rel(*pend_rel.pop(0))

    nc.compile()
    return nc


def _fold_weights(w_ih, w_hh, b_ih, b_hh, w_se, b_se, w_hp, b_hp):
    """Host-side constant folding. Gate order [i, f, g, o] (torch order)."""
    import ml_dtypes
    mf = ml_dtypes.bfloat16
    f = np.float32
    W_eff = w_hh + w_ih @ w_se @ w_hp                      # [4H, H]
    b_eff = (b_hp @ w_se.T + b_se) @ w_ih.T + b_ih + b_hh  # [4H]
    U = w_ih @ w_se                                        # [4H, 2]
    b1 = b_se @ w_ih.T + b_ih + b_hh                       # [4H]

    bias = np.stack(
        [b_eff[0:H], b_eff[H:2*H], b_eff[2*H:3*H], b_eff[3*H:4*H],
         b1[0:H], b1[H:2*H], b1[2*H:3*H], b1[3*H:4*H]], axis=1)  # [H, 8]
    return {
        "wg": np.ascontiguousarray(W_eff.T.astype(mf)),
        "whh": np.ascontiguousarray(w_hh.T.astype(mf)),
        "u": np.ascontiguousarray(U.T.astype(mf)),
        "bias": np.ascontiguousarray(bias, f),
        "whp": np.ascontiguousarray(w_hp.T.astype(mf)),
    }


def kernel(last_pos, last_pos_rel, h0, c0,
           w_ih, w_hh, b_ih, b_hh, w_se, b_se, w_hp, b_hp):
    import ml_dtypes
    mf = ml_dtypes.bfloat16
    b_hp = np.asarray(b_hp, np.float32)
    consts = _fold_weights(
        np.asarray(w_ih, np.float32), np.asarray(w_hh, np.float32),
        np.asarray(b_ih, np.float32), np.asarray(b_hh, np.float32),
        np.asarray(w_se, np.float32), np.asarray(b_se, np.float32),
        np.asarray(w_hp, np.float32), b_hp,
    )
    # host-side transpose + bf16 cast of the per-agent states
    h0T = np.ascontiguousarray(np.asarray(h0, np.float32).T.astype(mf))
    c0T = np.ascontiguousarray(np.asarray(c0, np.float32).T.astype(mf))
    lprT = np.ascontiguousarray(
        np.asarray(last_pos_rel, np.float32).T.astype(mf))

    npeds = h0T.shape[1]
    npc = npeds // NCORES
    if "nc" not in _CACHE or _CACHE.get("npc") != npc:
        _CACHE["nc"] = _build_program(npc)
        _CACHE["npc"] = npc
    nc = _CACHE["nc"]

    in_maps = []
    for ci in range(NCORES):
        cs = slice(ci * npc, (ci + 1) * npc)
        m = {"h0T": np.ascontiguousarray(h0T[:, cs]),
             "c0T": np.ascontiguousarray(c0T[:, cs]),
             "lprT": np.ascontiguousarray(lprT[:, cs])}
        m.update(consts)
        in_maps.append(m)

    from concourse.bass_utils import run_bass_kernel_spmd
    import os

    res = run_bass_kernel_spmd(
        nc, in_maps, list(range(NCORES)),
        tmpdir=os.environ.get("KERNEL_TRACE_DIR"),
    )
    _CACHE["exec_time_ns"] = res.exec_time_ns
    _CACHE["results"] = res

    out = np.empty((T, npeds, 2), np.float32)
    for ci in range(NCORES):
        rows = slice(ci * npc, (ci + 1) * npc)
        out[:, rows, 0] = np.asarray(res.results[ci]["outx"]) + b_hp[0]
        out[:, rows, 1] = np.asarray(res.results[ci]["outy"]) + b_hp[1]
    return out


# revision 30
# speedup vs baseline: 1.0002x; 1.0002x over previous
"""Trainium2 Bass kernel for the nn_Decoder LSTM-decoder problem.

Reference computation (per agent, 12 steps):
    gates = dec_in @ w_ih.T + h @ w_hh.T + (b_ih + b_hh)
    i, f, g, o = split(gates); c = sig(f)*c + sig(i)*tanh(g); h = sig(o)*tanh(c)
    rel = h @ w_hp.T + b_hp; dec_in = rel @ w_se.T + b_se
Output: rel per step, [12, N, 2].

Algebraic fusion: dec_in_t is linear in h_t, so for steps >= 2
    gates_t = h_{t-1} @ W_eff.T + b_eff,  W_eff = w_hh + w_ih @ w_se @ w_hp
and step 1 uses w_hh plus U = (w_ih @ w_se) applied to last_pos_rel.
last_pos is dead (never affects the output).

Distribution: pure data parallel, 8192 agents per core on 8 NeuronCores.

The Scalar engine (ACT) is the roofline: 5 LUT passes per (agent, hidden,
step) = 491520 FD-columns per core at 1 col/cycle @ 1.2 GHz (~410us) plus
~352 cycles fixed overhead per ACTIVATE. Design choices:
  - Units of 2048 agents; PSUM = 2 rotating slots of [128, 2048] fp32
    (4 banks each); gate ACT ops are FD=2048 (matmuls FD=512, the ISA cap).
  - Gates are processed [g, f, i, o] and the ACT stream per unit is
    [tanh_g, sig_f, sig_i, sig_o, tanh_c-half0, tanh_c-half1]: tanh_g
    first makes the m1/m2/c-add DVE chain finish early, and the cell
    update + tanh(c) run in FD-1024 halves so tanh_c-half0 is ready
    before sig_o retires -> the ACT engine streams with ~zero bubbles
    while the next unit's matmuls recycle the PSUM slots (important
    because the PE HAM clock-gate runs the PE at 1.2 GHz half the time).
  - rel = w_hp.T @ h is deferred two units and runs at the unit tail into
    the 5th PSUM alloc of the pool rotation; raw rel is copied to SBUF
    once and DMA'd to separate x/y DRAM planes; b_hp is added on the
    host, which also interleaves x/y into the [T, N, 2] output.
  - h0/c0/last_pos_rel are pre-transposed and bf16-cast on the HOST, so
    the prologue is 3 plain DMAs per unit (no on-device transposes).
"""

import sys

if "/opt/trn_rl_repo" not in sys.path:
    sys.path.insert(0, "/opt/trn_rl_repo")

import numpy as np

T = 12          # steps
H = 128         # hidden dim
NCORES = 8
NPC = 8192      # agents per core
CH = 2048       # agents per unit (one gate tile = 4 PSUM banks at fp32)

REL_COLTILE = False

_CACHE = {}


def _build_program(npc):
    import concourse.bass as bass
    import concourse.tile as tile
    from concourse import bacc, mybir

    dt = mybir.dt
    f32 = dt.float32
    bf16 = dt.bfloat16
    Act = mybir.ActivationFunctionType

    nsc = npc // CH
    assert npc % CH == 0

    nc = bacc.Bacc(
        "TRN2",
        target_bir_lowering=False,
        debug=False,
        num_devices=NCORES,
    )

    def din(name, shape, dt_=None):
        return nc.dram_tensor(
            name, list(shape), dt_ or f32, kind="ExternalInput"
        ).ap()

    # host-pretransposed bf16 states
    h0T_d = din("h0T", [H, npc], bf16)
    c0T_d = din("c0T", [H, npc], bf16)
    lprT_d = din("lprT", [2, npc], bf16)
    # lhsT layouts, K on partitions. Gate order [i, f, g, o] (torch order).
    wg_d = din("wg", [H, 4 * H], bf16)   # W_eff.T column blocks per gate
    whh_d = din("whh", [H, 4 * H], bf16)  # w_hh.T (step 1)
    u_d = din("u", [2, 4 * H], bf16)      # (w_ih @ w_se).T (step 1)
    bias_d = din("bias", [H, 8])          # ACT bias: [b_eff | b1] x [i,f,g,o]
    whp_d = din("whp", [H, 2], bf16)      # w_hp.T
    outx_d = nc.dram_tensor("outx", [T, npc], f32, kind="ExternalOutput").ap()
    outy_d = nc.dram_tensor("outy", [T, npc], f32, kind="ExternalOutput").ap()

    with tile.TileContext(nc) as tc:
        with (
            tc.tile_pool(name="wpool", bufs=1) as wp,
            tc.tile_pool(name="state", bufs=1) as state,
            tc.tile_pool(name="sig", bufs=2) as sigp,
            tc.tile_pool(name="tmp", bufs=2) as tmpp,
            tc.tile_pool(name="rels", bufs=2) as relp,
            tc.tile_pool(name="ps", bufs=2, space="PSUM") as psp,
        ):
            def wtile(ap, shape, tag, dt_=None):
                t_ = wp.tile(list(shape), dt_ or f32, tag=tag)
                nc.sync.dma_start(t_[:], ap)
                return t_

            wg = wtile(wg_d, [H, 4 * H], "wg", bf16)
            whh = wtile(whh_d, [H, 4 * H], "whh", bf16)
            u = wtile(u_d, [2, 4 * H], "u", bf16)
            bias = wtile(bias_d, [H, 8], "bias")
            whp = wtile(whp_d, [H, 2], "whp", bf16)

            h_sb = state.tile([H, npc], bf16, tag="h")
            c_sb = state.tile([H, npc], bf16, tag="c")
            lpr_sb = state.tile([2, npc], bf16, tag="lpr")

            units = [(t, sc) for t in range(T) for sc in range(nsc)]
            pend_back = []  # [(t, sc, so)] awaiting tanh_c + h update
            pend_rel = []   # [(t, sc)] awaiting rel matmul + writeback

            # PE warm-up: ~3.5us of junk matmuls (into a throwaway PSUM
            # tile, overwritten by the first real start=True matmul) so the
            # HAM clock-gate lifts to 2.4 GHz before step 0's gate matmuls.
            wu = psp.tile([128, CH], f32, tag="ps", name="warmup")
            for q in range(10):
                osl = slice((q % 4) * 512, (q % 4 + 1) * 512)
                nc.tensor.matmul(wu[:, osl], whh[:, 0:H], wg[:, 0:512],
                                 start=True, stop=True)

            def emit_rel(t_, sc_):
                """rel = w_hp.T @ h into the 5th PSUM alloc of this unit;
                col-tiled so one unit's rel occupies a single PSUM bank."""
                if REL_COLTILE:
                    rp = psp.tile([128, CH], f32, tag="ps",
                                  name=f"rel{t_}_{sc_}")
                    for q in range(4):
                        hs = slice(sc_ * CH + q * 512,
                                   sc_ * CH + (q + 1) * 512)
                        nc.tensor.matmul(
                            rp[32 * q:32 * q + 2, 0:512], whp[:],
                            h_sb[:, hs], start=True, stop=True,
                            tile_position=(0, 32 * q))
                    ex_x = relp.tile([4, 512], f32, tag="exx")
                    ex_y = relp.tile([4, 512], f32, tag="exy")
                    nc.vector.tensor_copy(ex_x[:], rp[0:97:32, 0:512])
                    nc.vector.tensor_copy(ex_y[:], rp[1:98:32, 0:512])
                    cols = slice(sc_ * CH, (sc_ + 1) * CH)
                    nc.sync.dma_start(outx_d[t_, cols], ex_x[:])
                    nc.sync.dma_start(outy_d[t_, cols], ex_y[:])
                else:
                    rp = psp.tile([128, CH], f32, tag="ps",
                                  name=f"rel{t_}_{sc_}")
                    for q in range(4):
                        osl = slice(q * 512, (q + 1) * 512)
                        hs = slice(sc_ * CH + q * 512,
                                   sc_ * CH + (q + 1) * 512)
                        nc.tensor.matmul(
                            rp[0:2, osl], whp[:], h_sb[:, hs],
                            start=True, stop=True)
                    ex = relp.tile([2, CH], f32, tag="ex")
                    nc.vector.tensor_copy(ex[:], rp[0:2, :])
                    cols = slice(sc_ * CH, (sc_ + 1) * CH)
                    nc.sync.dma_start(outx_d[t_, cols], ex[0:1, :])
                    nc.sync.dma_start(outy_d[t_, cols], ex[1:2, :])

            for u_idx, (t, sc) in enumerate(units):
                cols = slice(sc * CH, (sc + 1) * CH)
                first = t == 0

                if first:
                    nc.sync.dma_start(h_sb[:, cols], h0T_d[:, cols])
                    nc.sync.dma_start(c_sb[:, cols], c0T_d[:, cols])
                    nc.sync.dma_start(lpr_sb[:, cols], lprT_d[:, cols])

                # gate matmuls in processing order [g, f, i, o] (tanh gate
                # FIRST so the m2->cadd->tanh_c chain completes by unit end)
                gt = {}
                for g in (2, 1, 0, 3):
                    pt = psp.tile([128, CH], f32, tag="ps", name=f"g{g}")
                    wsl = slice(g * H, (g + 1) * H)
                    for q in range(4):
                        osl = slice(q * 512, (q + 1) * 512)
                        hs = slice(sc * CH + q * 512,
                                   sc * CH + (q + 1) * 512)
                        if first:
                            nc.tensor.matmul(
                                pt[:, osl], whh[:, wsl], h_sb[:, hs],
                                start=True, stop=False)
                            nc.tensor.matmul(
                                pt[:, osl], u[:, wsl], lpr_sb[:, hs],
                                start=False, stop=True)
                        else:
                            nc.tensor.matmul(
                                pt[:, osl], wg[:, wsl], h_sb[:, hs],
                                start=True, stop=True)
                    gt[g] = pt

                # gate activations (bias fused; cols 4..7 hold step-1 biases)
                bcol = 4 if first else 0
                tg = sigp.tile([128, CH], bf16, tag="tg")
                sf = sigp.tile([128, CH], bf16, tag="sf")
                si = sigp.tile([128, CH], bf16, tag="si")
                so = sigp.tile([128, CH], bf16, tag="so")
                nc.scalar.activation(tg[:], gt[2][:], Act.Tanh,
                                     bias=bias[:, bcol + 2:bcol + 3])
                nc.scalar.activation(sf[:], gt[1][:], Act.Sigmoid,
                                     bias=bias[:, bcol + 1:bcol + 2])
                nc.scalar.activation(si[:], gt[0][:], Act.Sigmoid,
                                     bias=bias[:, bcol:bcol + 1])
                nc.scalar.activation(so[:], gt[3][:], Act.Sigmoid,
                                     bias=bias[:, bcol + 3:bcol + 4])

                # DVE cell update in FD-1024 halves so the first tanh(c)
                # half is ready before sig_o finishes -> gapless ACT stream.
                m1 = tmpp.tile([128, CH], bf16, tag="m1")
                m2 = tmpp.tile([128, CH], bf16, tag="m2")
                tcl = sigp.tile([128, CH], bf16, tag="tc")
                for hf in range(2):
                    hsl = slice(hf * 1024, (hf + 1) * 1024)
                    csl = slice(sc * CH + hf * 1024, sc * CH + (hf + 1) * 1024)
                    nc.vector.tensor_mul(m1[:, hsl], sf[:, hsl],
                                         c_sb[:, csl])
                    nc.vector.tensor_mul(m2[:, hsl], si[:, hsl], tg[:, hsl])
                    nc.vector.tensor_add(c_sb[:, csl], m1[:, hsl],
                                         m2[:, hsl])
                    nc.scalar.activation(tcl[:, hsl], c_sb[:, csl], Act.Tanh)
                pend_back.append((t, sc, so, tcl))

                # rel for the unit two back (h final; slot free after tanh_g)
                pend_rel.append((t, sc))
                if len(pend_rel) > 2:
                    emit_rel(*pend_rel.pop(0))

                # h update (deferred DVE mul; h_prev consumers are >=2
                # units away so this can run late without stalling anyone)
                if len(pend_back) > 1:
                    t_, sc_, so_, tcl_ = pend_back.pop(0)
                    pcols = slice(sc_ * CH, (sc_ + 1) * CH)
                    nc.vector.tensor_mul(h_sb[:, pcols], so_[:], tcl_[:])

                # in the final unit, drain one extra rel now that its h is
                # final -- shortens the serial epilogue
                if u_idx == len(units) - 1:
                    emit_rel(*pend_rel.pop(0))

            while pend_back:
                t_, sc_, so_, tcl_ = pend_back.pop(0)
                pcols = slice(sc_ * CH, (sc_ + 1) * CH)
                nc.vector.tensor_mul(h_sb[:, pcols], so_[:], tcl_[:])
            while pend_rel:
                emit_rel(*pend_rel.pop(0))

    nc.compile()
    return nc


def _fold_weights(w_ih, w_hh, b_ih, b_hh, w_se, b_se, w_hp, b_hp):
    """Host-side constant folding. Gate order [i, f, g, o] (torch order)."""
    import ml_dtypes
    mf = ml_dtypes.bfloat16
    f = np.float32
    W_eff = w_hh + w_ih @ w_se @ w_hp                      # [4H, H]
    b_eff = (b_hp @ w_se.T + b_se) @ w_ih.T + b_ih + b_hh  # [4H]
    U = w_ih @ w_se                                        # [4H, 2]
    b1 = b_se @ w_ih.T + b_ih + b_hh                       # [4H]

    bias = np.stack(
        [b_eff[0:H], b_eff[H:2*H], b_eff[2*H:3*H], b_eff[3*H:4*H],
         b1[0:H], b1[H:2*H], b1[2*H:3*H], b1[3*H:4*H]], axis=1)  # [H, 8]
    return {
        "wg": np.ascontiguousarray(W_eff.T.astype(mf)),
        "whh": np.ascontiguousarray(w_hh.T.astype(mf)),
        "u": np.ascontiguousarray(U.T.astype(mf)),
        "bias": np.ascontiguousarray(bias, f),
        "whp": np.ascontiguousarray(w_hp.T.astype(mf)),
    }


def kernel(last_pos, last_pos_rel, h0, c0,
           w_ih, w_hh, b_ih, b_hh, w_se, b_se, w_hp, b_hp):
    import ml_dtypes
    mf = ml_dtypes.bfloat16
    b_hp = np.asarray(b_hp, np.float32)
    consts = _fold_weights(
        np.asarray(w_ih, np.float32), np.asarray(w_hh, np.float32),
        np.asarray(b_ih, np.float32), np.asarray(b_hh, np.float32),
        np.asarray(w_se, np.float32), np.asarray(b_se, np.float32),
        np.asarray(w_hp, np.float32), b_hp,
    )
    # host-side transpose + bf16 cast of the per-agent states
    h0T = np.ascontiguousarray(np.asarray(h0, np.float32).T.astype(mf))
    c0T = np.ascontiguousarray(np.asarray(c0, np.float32).T.astype(mf))
    lprT = np.ascontiguousarray(
        np.asarray(last_pos_rel, np.float32).T.astype(mf))

    npeds = h0T.shape[1]
    npc = npeds // NCORES
    if "nc" not in _CACHE or _CACHE.get("npc") != npc:
        _CACHE["nc"] = _build_program(npc)
        _CACHE["npc"] = npc
    nc = _CACHE["nc"]

    in_maps = []
    for ci in range(NCORES):
        cs = slice(ci * npc, (ci + 1) * npc)
        m = {"h0T": np.ascontiguousarray(h0T[:, cs]),
             "c0T": np.ascontiguousarray(c0T[:, cs]),
             "lprT": np.ascontiguousarray(lprT[:, cs])}
        m.update(consts)
        in_maps.append(m)

    from concourse.bass_utils import run_bass_kernel_spmd
    import os

    res = run_bass_kernel_spmd(
        nc, in_maps, list(range(NCORES)),
        tmpdir=os.environ.get("KERNEL_TRACE_DIR"),
    )
    _CACHE["exec_time_ns"] = res.exec_time_ns
    _CACHE["results"] = res

    out = np.empty((T, npeds, 2), np.float32)
    for ci in range(NCORES):
        rows = slice(ci * npc, (ci + 1) * npc)
        out[:, rows, 0] = np.asarray(res.results[ci]["outx"]) + b_hp[0]
        out[:, rows, 1] = np.asarray(res.results[ci]["outy"]) + b_hp[1]
    return out
